# revision 23
# baseline (speedup 1.0000x reference)
"""Multi-head attention block (QKV proj + softmax attention + out proj) on 8
Trainium2 NeuronCores.

Problem shapes: x [4, 1024, 1024], Wqkv [3072, 1024], bqkv [3072],
W1 [1024, 1024], b1 [1024].  out = Attention(x) @ W1.T + b1, 16 heads, d=64,
softmax scale 1/sqrt(1024) = 1/32.

Sharding: core c handles batch b = c // 2 and head-group hg = c % 2 (8 of the
16 heads).  Each core computes its heads' QKV projection, full attention for
those heads over its batch, and a *partial* output projection against the
W1 columns its heads feed.  The host sums the two partials per batch and adds
b1.  No device collectives.

Layout trick: the host feeds per-core inputs pre-transposed (x.T, Wqkv_loc.T,
W1_loc.T) so every matmul operand lands in SBUF with its contraction dim on
partitions via plain contiguous DMAs — no on-chip transposes anywhere:
  - Q.T computed as [feat, tok]; K.T stored twice as zero-padded parity
    copies (ktp0 = [K_even ; 0], ktp1 = [0 ; K_odd]) so every S matmul is a
    full 128-row *untiled* matmul (the zero rows contribute nothing).  This
    avoids the 64-row tile-mode switches that otherwise drain the PE between
    every PV/S group.
  - V computed as [tok, feat] (lhsT=x.T chunk, rhs=W.T v-cols), stored with a
    ones column appended per head so the PV matmul also produces row sums l_i
  - S.T[j, i] = Kpad.T_chunk.T @ Q.T per head; exp on ScalarE (dots are
    bounded, no max subtraction needed); PV accumulates out.T[e, i] over
    j-chunks with the 65th lhsT column giving l_i; normalize by gpsimd
    partition-broadcast of 1/l; A.T accumulates in [feat, tok] layout which
    feeds the final projection directly.  Final out is partial.T [outdim, tok].

Schedule: input DMAs stream on 3 queues chunk-paced; 6 warmup matmuls
(scratch memset on gpsimd, which boots earliest) ramp the PE clock before
chunk 0 lands.  Attention runs in 8 slots (pair x token-half), PV one slot
behind S/exp; V-projection, the remaining QK pairs, and the th=0 half of the
output projection interleave as per-slot filler.  In the last slot, four
th=1 out-proj groups accumulate as column-pairs of two wide psS tiles the
final exps free, overlapping the last norm chain; only 11 matmuls remain
after the final norm.  Matmuls run in bf16 (fp32 PSUM accumulation).
"""

import numpy as np

B = 4
N = 1024            # tokens per batch
DIM = 1024          # model dim
HEADS = 16
D = DIM // HEADS    # 64
NCORES = 8
HG = 2              # head groups (tensor-parallel degree over heads)
NHL = HEADS // HG   # 8 local heads
FQ = NHL * D        # 512 local q (or k or v) features
FT = 3 * FQ         # 1536 local qkv features
P = 128
TH = 512            # token half (matmul free dim)

_CACHE = {}


def _build():
    from contextlib import ExitStack

    import concourse.bacc as bacc
    import concourse.bass as bass
    import concourse.tile as tile
    from concourse import mybir

    f32 = mybir.dt.float32
    mmdt = mybir.dt.bfloat16

    nc = bacc.Bacc("TRN2", target_bir_lowering=False)

    xT = nc.dram_tensor("xT", [DIM, N], mmdt, kind="ExternalInput")
    wqkvT = nc.dram_tensor("wqkvT", [DIM, FT], mmdt, kind="ExternalInput")
    bqkvT = nc.dram_tensor("bqkvT", [P, FT // P], f32, kind="ExternalInput")
    bv = nc.dram_tensor("bv", [FQ], f32, kind="ExternalInput")
    w1T = nc.dram_tensor("w1T", [FQ, DIM], mmdt, kind="ExternalInput")
    outdt = mmdt
    outT = nc.dram_tensor("outT", [DIM, N], outdt, kind="ExternalOutput")

    Exp = mybir.ActivationFunctionType.Exp
    Copy = mybir.ActivationFunctionType.Copy

    with tile.TileContext(nc) as tc, ExitStack() as ctx:
        const = ctx.enter_context(tc.tile_pool(name="const", bufs=1))
        psS = ctx.enter_context(tc.tile_pool(name="psS", bufs=2, space="PSUM"))
        psP = ctx.enter_context(tc.tile_pool(name="psP", bufs=2, space="PSUM"))
        psB = ctx.enter_context(tc.tile_pool(name="psB", bufs=2, space="PSUM"))
        outp = ctx.enter_context(tc.tile_pool(name="outp", bufs=4))
        small = ctx.enter_context(tc.tile_pool(name="small", bufs=4))
        loadp = ctx.enter_context(tc.tile_pool(name="loadp", bufs=1))
        ptp = ctx.enter_context(tc.tile_pool(name="ptp", bufs=4))

        # persistent SBUF
        qt = const.tile([P, 4, N], mmdt)        # Q.T  [f-inner, pair, tok]
        ktp0 = const.tile([P, 4, N], mmdt)      # K.T even heads at rows 0:64
        ktp1 = const.tile([P, 4, N], mmdt)      # K.T odd heads at rows 64:128
        vs = const.tile([P, 8, NHL * 65], mmdt)  # V'  [tok-inner, j-chunk, h*65+e]
        at = const.tile([P, 4, N], mmdt)        # A.T  [f-inner, f-chunk, tok]
        w1s = const.tile([P, 4, DIM], mmdt)     # W1loc.T [f-inner, f-chunk, out]
        bqv = const.tile([P, FT // P], f32)     # qkv bias, per-partition per f-block
        bvb = const.tile([P, FQ], f32)          # v bias broadcast across partitions

        # warmup scratch: memset on gpsimd (its queue drains framework init
        # earliest) so PE warmup matmuls can start the p-state ramp while the
        # first input chunk is still streaming in
        wmup = const.tile([P, TH], mmdt)
        nc.gpsimd.memset(wmup, 0.5)

        xT_r = xT.ap().rearrange("(c p) t -> p c t", p=P)
        wT_r = wqkvT.ap().rearrange("(c p) f -> p c f", p=P)
        xs = []
        ws = []
        wvs = []
        for c in range(8):
            xs.append(loadp.tile([P, N], mmdt, name=f"xs{c}"))
            ws.append(loadp.tile([P, 2 * FQ], mmdt, name=f"ws{c}"))
            wvs.append(loadp.tile([P, FQ], mmdt, name=f"wvs{c}"))
        # --- input DMA stream: 3 queues, transfers round-robined in
        # consumption order (w_c qk-cols + x_c per chunk), v-cols next,
        # w1/bvb last.  Separate tiles per DMA so nothing waits on a
        # later write to the same tile. ---
        queues = [nc.sync, nc.scalar, nc.gpsimd]
        qi = [0]

        def q_dma(out, in_):
            queues[qi[0] % 3].dma_start(out=out, in_=in_)
            qi[0] += 1

        for c in range(8):
            if c < 4:
                # split the first chunks so the earliest-needed transfers
                # aren't stuck time-sharing DMA bandwidth with the flood
                q_dma(ws[c][:, 0:FQ], wT_r[:, c, 0:FQ])
                q_dma(ws[c][:, FQ:2 * FQ], wT_r[:, c, FQ:2 * FQ])
                q_dma(xs[c][:, 0:TH], xT_r[:, c, 0:TH])
                q_dma(xs[c][:, TH:N], xT_r[:, c, TH:N])
                if c == 1:
                    # tiny bias transfer slotted behind the first two chunks:
                    # early enough for the bias adds, off the chunk-0 path
                    q_dma(bqv, bqkvT.ap())
            else:
                q_dma(ws[c], wT_r[:, c, 0:2 * FQ])
                q_dma(xs[c], xT_r[:, c])
        for c in range(8):
            q_dma(wvs[c], wT_r[:, c, 2 * FQ:FT])
        bv_bc = bass.AP(
            tensor=bv.ap().tensor,
            offset=0,
            ap=[[0, P], [1, FQ]],
        )
        q_dma(bvb, bv_bc)
        w1_r = w1T.ap().rearrange("(c p) o -> p c o", p=P)
        q_dma(w1s[:, 0:2], w1_r[:, 0:2])
        q_dma(w1s[:, 2:4], w1_r[:, 2:4])
        # ones column of V' (row sums in the PV matmul)
        nc.vector.memset(
            vs.rearrange("p c (h e) -> p c h e", e=65)[:, :, :, 64:65],
            1.0,
        )
        # zero halves of the padded K.T parity copies (one-time, during the
        # input stream while the vector engine is otherwise idle)
        nc.vector.memset(ktp0[D:P, :, :], 0.0)
        nc.vector.memset(ktp1[0:D, :, :], 0.0)

        # ---- QK projection for pairs 0,1 -- c-major so each chunk's matmuls
        # fire as the chunk lands; 8 psum accumulation groups live at once ----
        q0 = psS.tile([P, N], f32, tag="ps", name="q0g")
        k0 = psS.tile([P, N], f32, tag="ps", name="k0g")
        q1 = [psB.tile([P, TH], f32, tag="pv", name=f"q1g{t}") for t in range(2)]
        k1 = [psP.tile([P, TH], f32, tag="pp", name=f"k1g{t}") for t in range(2)]
        # p-state warmup: dummy matmuls on scratch data while the first input
        # chunk streams in, so real matmuls start near full clock.  Results
        # land in q1[0], which the first real accumulation resets (start=True).
        for _ in range(4):
            nc.tensor.matmul(q1[0], wmup[:, 0:P], wmup, start=True, stop=True)
        for c in range(8):
            st = dict(start=(c == 0), stop=(c == 7))
            wc = ws[c]
            for th in range(2):
                sl = slice(th * TH, (th + 1) * TH)
                nc.tensor.matmul(q0[:, sl], wc[:, 0:P], xs[c][:, sl], **st)
            for th in range(2):
                sl = slice(th * TH, (th + 1) * TH)
                nc.tensor.matmul(k0[:, sl], wc[:, 4 * P:5 * P], xs[c][:, sl], **st)
            for th in range(2):
                sl = slice(th * TH, (th + 1) * TH)
                nc.tensor.matmul(q1[th], wc[:, P:2 * P], xs[c][:, sl], **st)
            for th in range(2):
                sl = slice(th * TH, (th + 1) * TH)
                nc.tensor.matmul(k1[th], wc[:, 5 * P:6 * P], xs[c][:, sl], **st)

        def k_add(fb, src_lo, src_hi, sl):
            # k bias add, split per parity into the zero-padded copies
            p_ = fb % 4
            nc.vector.tensor_scalar_add(
                out=ktp0[0:D, p_, sl], in0=src_lo, scalar1=bqv[0:D, fb:fb + 1])
            nc.vector.tensor_scalar_add(
                out=ktp1[D:P, p_, sl], in0=src_hi, scalar1=bqv[D:P, fb:fb + 1])

        # bias adds on vector (gpsimd can't read PSUM), ordered so the first
        # S matmul's exact inputs land first: even-parity K.T pair0 (both
        # halves), then the ih=0 half of Q.T pair0; everything else trails.
        Ident = mybir.ActivationFunctionType.Identity
        for th in range(2):
            sl = slice(th * TH, (th + 1) * TH)
            nc.vector.tensor_scalar_add(
                out=ktp0[0:D, 0, sl], in0=k0[0:D, sl], scalar1=bqv[0:D, 4:5])
            # odd-parity evacuation on scalar (idle until the first exp) to
            # halve the serial bias chain gating the first S matmul
            nc.scalar.activation(
                out=ktp1[D:P, 0, sl], in_=k0[D:P, sl], func=Ident,
                bias=bqv[D:P, 4:5], scale=1.0)
        nc.vector.tensor_scalar_add(
            out=qt[:, 0, 0:TH], in0=q0[:, 0:TH], scalar1=bqv[:, 0:1])
        nc.vector.tensor_scalar_add(
            out=qt[:, 0, TH:N], in0=q0[:, TH:N], scalar1=bqv[:, 0:1])
        for th in range(2):
            sl = slice(th * TH, (th + 1) * TH)
            nc.vector.tensor_scalar_add(
                out=ktp0[0:D, 1, sl], in0=k1[th][0:D, :], scalar1=bqv[0:D, 5:6])
            nc.scalar.activation(
                out=ktp1[D:P, 1, sl], in_=k1[th][D:P, :], func=Ident,
                bias=bqv[D:P, 5:6], scale=1.0)
            nc.vector.tensor_scalar_add(
                out=qt[:, 1, sl], in0=q1[th], scalar1=bqv[:, 1:2])

        # ---- filler generators (emitted mid-slot to keep the PE dense) ----
        def qk_half(fb):
            # one f-block (128 features) of a remaining pair, on psP
            ps0 = psP.tile([P, TH], f32, tag="pp", name=f"q{fb}a")
            ps1 = psP.tile([P, TH], f32, tag="pp", name=f"q{fb}b")
            pstiles = (ps0, ps1)
            for c in range(8):
                for th in range(2):
                    nc.tensor.matmul(
                        pstiles[th],
                        ws[c][:, fb * P:(fb + 1) * P],
                        xs[c][:, th * TH:(th + 1) * TH],
                        start=(c == 0),
                        stop=(c == 7),
                    )
            for th in range(2):
                sl = slice(th * TH, (th + 1) * TH)
                if fb < 4:
                    nc.vector.tensor_scalar_add(
                        out=qt[:, fb % 4, sl],
                        in0=pstiles[th],
                        scalar1=bqv[:, fb:fb + 1],
                    )
                else:
                    k_add(fb, pstiles[th][0:D, :], pstiles[th][D:P, :], sl)

        def v_proj(jc):
            # V columns for j-chunk jc: psum[tok 128, f 512] on psP
            pv = psP.tile([P, TH], f32, tag="pp", name=f"v{jc}")
            for c in range(8):
                nc.tensor.matmul(
                    pv,
                    xs[c][:, jc * P:(jc + 1) * P],
                    wvs[c],
                    start=(c == 0),
                    stop=(c == 7),
                )
            nc.vector.tensor_add(
                out=vs[:, jc].rearrange("p (h e) -> p h e", e=65)[:, :, 0:64],
                in0=pv.rearrange("p (h e) -> p h e", e=64),
                in1=bvb.rearrange("p (h e) -> p h e", e=64),
            )

        dma_engines = [nc.sync, nc.gpsimd]
        outT_r = outT.ap().rearrange("(b p) t -> p b t", p=P)
        gctr = [0]

        def og_add(ob, th, fps, fcs):
            for fc in fcs:
                nc.tensor.matmul(
                    fps,
                    w1s[:, fc, ob * P:(ob + 1) * P],
                    at[:, fc, th * TH:(th + 1) * TH],
                    start=(fc == 0),
                    stop=(fc == 3),
                )

        def og_start(ob, th, pool, tag, fcs, wide=False):
            shape = [P, N] if wide else [P, TH]
            fps = pool.tile(shape, f32, tag=tag, name=f"f{ob}_{th}")
            if wide:
                fps = fps[:, 0:TH]
            og_add(ob, th, fps, fcs)
            return fps

        def og_finish(ob, th, fps, fcs, cast_scalar=False, split_dma=False,
                      dma_q=None):
            og_add(ob, th, fps, fcs)
            g = gctr[0]
            gctr[0] += 1
            ot = outp.tile([P, TH], outdt, tag="ot")
            if cast_scalar:
                nc.scalar.activation(out=ot, in_=fps, func=Copy, scale=1.0)
            else:
                nc.vector.tensor_copy(out=ot, in_=fps)
            if split_dma:
                # last group: halve the final transfer the end-of-kernel
                # barrier waits on by using both queues (off gpsimd, whose
                # end-of-program drain is the slowest)
                hw_ = TH // 2
                for h2, eng in enumerate((nc.sync, nc.scalar)):
                    eng.dma_start(
                        out=outT_r[:, ob,
                                   th * TH + h2 * hw_:th * TH + (h2 + 1) * hw_],
                        in_=ot[:, h2 * hw_:(h2 + 1) * hw_],
                    )
            else:
                eng = dma_q if dma_q is not None else dma_engines[g % 2]
                eng.dma_start(
                    out=outT_r[:, ob, th * TH:(th + 1) * TH], in_=ot
                )

        def final_group(ob, th, pool, tag, cast_scalar=False, split_dma=False):
            og_finish(ob, th, og_start(ob, th, pool, tag, []), range(4),
                      cast_scalar, split_dma)

        # wide-pair out-proj group: two output blocks share one [P, N] psS
        # tile as column halves; one wide cast, two DMAs
        def og_pair_start(oba, obb, th, fcs):
            fps = psS.tile([P, N], f32, tag="ps", name=f"w{oba}{obb}")
            og_add(oba, th, fps[:, 0:TH], fcs)
            og_add(obb, th, fps[:, TH:N], fcs)
            return fps

        def og_pair_finish(oba, obb, th, fps, fcs, engines=None):
            og_add(oba, th, fps[:, 0:TH], fcs)
            og_add(obb, th, fps[:, TH:N], fcs)
            ow = outp.tile([P, N], outdt, tag="otw", bufs=2)
            nc.vector.tensor_copy(out=ow, in_=fps)
            tsl = slice(th * TH, (th + 1) * TH)
            e0, e1 = engines if engines is not None else dma_engines
            e0.dma_start(out=outT_r[:, oba, tsl], in_=ow[:, 0:TH])
            e1.dma_start(out=outT_r[:, obb, tsl], in_=ow[:, TH:N])

        # ---- attention slots.  s_exp emits S+exp for (p_, ih, jcp); pv
        # emits the PV accumulation for one head over a jc pair ----
        pts = {}

        def s_exp(p_, ih, jcp):
            isl = slice(ih * TH, (ih + 1) * TH)
            if jcp == 0:
                pts[(p_, ih, 0)] = ptp.tile(
                    [P, 8, TH], mmdt, tag="pt", name=f"pt{p_}_{ih}e")
                pts[(p_, ih, 1)] = ptp.tile(
                    [P, 8, TH], mmdt, tag="pt", name=f"pt{p_}_{ih}o")
            se = psS.tile([P, N], f32, tag="ps", name=f"se{p_}_{ih}_{jcp}")
            so = psS.tile([P, N], f32, tag="ps", name=f"so{p_}_{ih}_{jcp}")
            for k in range(2):
                jc = 2 * jcp + k
                ksl = slice(k * TH, (k + 1) * TH)
                # full 128-row untiled matmuls against the zero-padded K.T
                # parity copies: no PE tile-mode switches anywhere
                nc.tensor.matmul(
                    se[:, ksl],
                    ktp0[:, p_, jc * P:(jc + 1) * P],
                    qt[:, p_, isl],
                    start=True, stop=True,
                )
                nc.tensor.matmul(
                    so[:, ksl],
                    ktp1[:, p_, jc * P:(jc + 1) * P],
                    qt[:, p_, isl],
                    start=True, stop=True,
                )
            nc.scalar.activation(
                out=pts[(p_, ih, 0)][:, 2 * jcp:2 * jcp + 2, :], in_=se,
                func=Exp, scale=1.0 / 32.0)
            nc.scalar.activation(
                out=pts[(p_, ih, 1)][:, 2 * jcp:2 * jcp + 2, :], in_=so,
                func=Exp, scale=1.0 / 32.0)

        def pv_mm(p_, ih, hh, ot_, jcp):
            h = 2 * p_ + hh
            pt = pts[(p_, ih, hh)]
            for k in range(2):
                jc = 2 * jcp + k
                nc.tensor.matmul(
                    ot_[0:65],
                    vs[:, jc, h * 65:h * 65 + 65],
                    pt[:, jc, :],
                    start=(jc == 0),
                    stop=(jc == 7),
                )

        def norm_rec(ot_):
            # l-row copy + reciprocal + partition broadcast; returns the
            # broadcast tile for norm_mul
            lrow = small.tile([1, TH], f32, tag="lrow")
            nc.vector.tensor_copy(out=lrow, in_=ot_[64:65, :])
            rec = small.tile([1, TH], f32, tag="rec")
            nc.vector.reciprocal_approx_fast(out=rec, in_=lrow)
            bc = small.tile([D, TH], f32, tag="bc")
            nc.gpsimd.partition_broadcast(out_ap=bc, in_ap=rec)
            return bc

        def norm_mul(p_, ih, hh, ot_, bc):
            pb = hh * D
            isl = slice(ih * TH, (ih + 1) * TH)
            nc.vector.tensor_mul(
                out=at[pb:pb + D, p_, isl],
                in0=ot_[0:64, :],
                in1=bc,
            )
            del pts[(p_, ih, hh)]

        def norm(p_, ih, hh, ot_):
            norm_mul(p_, ih, hh, ot_, norm_rec(ot_))

        # slot sequence: (pair, ih); PV runs one slot behind S/exp.
        # Emission-order constraints (the dep tracker is program-order):
        #   V before slot (1,0)'s pv; qk_half(p)/(p+4) before slot (p,0)'s S;
        #   any og matmul reading at[:, fc, th] only after norm(fc, th) was
        #   emitted — norm(p, ih) lands at the END of the following slot.
        slots = [(0, 0), (1, 0), (2, 0), (3, 0), (0, 1), (1, 1), (2, 1), (3, 1)]
        ogs = {}

        def og_stage(ob, fcs, fin=False):
            if ob not in ogs:
                ogs[ob] = og_start(ob, 0, psP, "pp", fcs)
            elif fin:
                og_finish(ob, 0, ogs[ob], fcs)
            else:
                og_add(ob, 0, ogs[ob], fcs)

        fillers = {
            (0, 0): lambda: (v_proj(0), v_proj(1)),
            (0, 1): lambda: (v_proj(2), v_proj(3)),
            (0, 2): lambda: (v_proj(4), v_proj(5)),
            (0, 3): lambda: (v_proj(6), v_proj(7)),
            (1, 0): lambda: qk_half(2), (1, 2): lambda: qk_half(6),
            (2, 0): lambda: qk_half(3), (2, 2): lambda: qk_half(7),
            # th0 out-proj staged as norms land: pairs 0,1 usable from si3,
            # pair2 from si4, pair3 from si5
            (3, 1): lambda: og_stage(0, [0, 1]),
            (3, 2): lambda: og_stage(1, [0, 1]),
            (4, 1): lambda: og_stage(0, [2]),
            (4, 2): lambda: og_stage(1, [2]),
            (5, 0): lambda: og_stage(0, [3], fin=True),
            (5, 1): lambda: og_stage(1, [3], fin=True),
            (5, 2): lambda: final_group(2, 0, psP, "pp"),
            (5, 3): lambda: final_group(3, 0, psP, "pp"),
            (6, 0): lambda: final_group(4, 0, psP, "pp"),
            (6, 1): lambda: final_group(5, 0, psP, "pp"),
            (6, 2): lambda: final_group(6, 0, psP, "pp"),
            (6, 3): lambda: final_group(7, 0, psP, "pp"),
        }

        # Emission order per slot staggers S(jcp) behind independent work
        # (prev-slot PV, fillers) so the in-order PE queue never head-of-line
        # blocks on exp(jcp-1) freeing the S psum buffer.
        prev = None
        prev_o = None
        for si, (p_, ih) in enumerate(slots[:-1]):
            oe = psB.tile([P, TH], f32, tag="pv", name=f"oe{p_}_{ih}")
            oo = psB.tile([P, TH], f32, tag="pv", name=f"oo{p_}_{ih}")

            def fill(j):
                f = fillers.get((si, j))
                if f is not None:
                    f()

            if si == 0:
                # cover the qk bias-add latency with V work before the
                # first S matmul
                fill(0)
            # PV for the previous slot is front-loaded into jcp 0-1 (all its
            # exps finished last slot) and its norm emitted mid-slot, so the
            # norm's serial vector/gpsimd chain completes well before the
            # NEXT slot's PV needs the psB banks it frees.
            for jcp in range(4):
                if jcp > 0:
                    fill(jcp - 1 if si > 0 else jcp)
                if prev is not None and jcp < 2:
                    pp, pih = prev
                    pv_mm(pp, pih, 0, prev_o[0], 2 * jcp)
                    pv_mm(pp, pih, 1, prev_o[1], 2 * jcp)
                    pv_mm(pp, pih, 0, prev_o[0], 2 * jcp + 1)
                    pv_mm(pp, pih, 1, prev_o[1], 2 * jcp + 1)
                s_exp(p_, ih, jcp)
                if jcp == 1 and prev is not None:
                    pp, pih = prev
                    norm(pp, pih, 0, prev_o[0])
                    norm(pp, pih, 1, prev_o[1])
            if si > 0:
                fill(3)
            prev = (p_, ih)
            prev_o = (oe, oo)

        # ---- last slot (3,1): software-pipelined, PV fused in-slot so the
        # tail after the final exp is as short as possible.  Four extra th=1
        # out-proj groups (ob2-5) accumulate fc 0-2 as column-pairs of the
        # wide psS tiles the last exps free, overlapping the final norm
        # chain; after the norm only fc=3 finishers and two full groups
        # remain. ----
        p_, ih = slots[-1]
        oe = psB.tile([P, TH], f32, tag="pv", name="oe31")
        oo = psB.tile([P, TH], f32, tag="pv", name="oo31")
        pre = {}
        for jcp in range(4):
            s_exp(p_, ih, jcp)
            if jcp == 0:
                pp, pih = prev
                for j2 in range(4):
                    pv_mm(pp, pih, 0, prev_o[0], j2)
                    pv_mm(pp, pih, 1, prev_o[1], j2)
                norm(pp, pih, 0, prev_o[0])
                norm(pp, pih, 1, prev_o[1])
            else:
                pv_mm(p_, ih, 0, oe, jcp - 1)
                pv_mm(p_, ih, 1, oo, jcp - 1)
            if jcp == 2:
                pre[0] = og_start(0, 1, psP, "pp", [0, 1, 2])
            if jcp == 3:
                pre[1] = og_start(1, 1, psP, "pp", [0, 1, 2])
        # wideA rides on the psS slot freed by the first jcp=3 exp, covering
        # the final exp latency.  Both heads' reciprocal chains are emitted
        # before either multiply so they interleave on vector/gpsimd, and
        # wB's matmuls fill the PE during the broadcast latency.
        def norm_rec_s(ot_):
            # tail-only variant: l-row copy on scalar (idle after the last
            # exp) so both heads' reciprocal chains overlap fully
            lrow = small.tile([1, TH], f32, tag="lrow")
            nc.scalar.activation(out=lrow, in_=ot_[64:65, :], func=Copy,
                                 scale=1.0)
            rec = small.tile([1, TH], f32, tag="rec")
            nc.vector.reciprocal_approx_fast(out=rec, in_=lrow)
            bc = small.tile([D, TH], f32, tag="bc")
            nc.gpsimd.partition_broadcast(out_ap=bc, in_ap=rec)
            return bc

        pv_mm(p_, ih, 0, oe, 3)
        bc0 = norm_rec_s(oe)
        wA = og_pair_start(2, 3, 1, [0, 1, 2])
        pv_mm(p_, ih, 1, oo, 3)
        bc1 = norm_rec_s(oo)
        wB = og_pair_start(4, 5, 1, [0, 1, 2])
        norm_mul(p_, ih, 0, oe, bc0)
        norm_mul(p_, ih, 1, oo, bc1)
        og7 = og_start(7, 1, psB, "pv", [0, 1, 2])
        # tail: fc=3 finishers (ob0 first so its cast frees psP for ob6),
        # casts split across vector+scalar; last DMAs off gpsimd so its
        # slow end-of-program drain starts early
        og_finish(0, 1, pre[0], [3], cast_scalar=True, dma_q=nc.sync)
        og_pair_finish(2, 3, 1, wA, [3], engines=(nc.sync, nc.scalar))
        og6 = og_start(6, 1, psP, "pp", [0, 1, 2])
        og_finish(1, 1, pre[1], [3], cast_scalar=True, dma_q=nc.sync)
        og_pair_finish(4, 5, 1, wB, [3], engines=(nc.sync, nc.scalar))
        og_finish(7, 1, og7, [3], cast_scalar=True, dma_q=nc.scalar)
        og_finish(6, 1, og6, [3], cast_scalar=False, split_dma=True)

    nc.finalize()
    return nc


def _get_nc():
    if "nc" not in _CACHE:
        _CACHE["nc"] = _build()
    return _CACHE["nc"]


def make_in_maps(x, Wqkv, bqkv, W1):
    import ml_dtypes
    mmnp = ml_dtypes.bfloat16
    x = np.ascontiguousarray(np.asarray(x, dtype=np.float32))
    Wqkv = np.asarray(Wqkv, dtype=np.float32)
    bqkv = np.asarray(bqkv, dtype=np.float32)
    W1 = np.asarray(W1, dtype=np.float32)
    in_maps = []
    for c in range(NCORES):
        b, hg = divmod(c, HG)
        qsl = slice(hg * FQ, (hg + 1) * FQ)
        ksl = slice(DIM + hg * FQ, DIM + (hg + 1) * FQ)
        vsl = slice(2 * DIM + hg * FQ, 2 * DIM + (hg + 1) * FQ)
        w_loc = np.concatenate([Wqkv[qsl], Wqkv[ksl], Wqkv[vsl]], axis=0)
        b_loc = np.concatenate([bqkv[qsl], bqkv[ksl], bqkv[vsl]])
        in_maps.append({
            "xT": np.ascontiguousarray(x[b].T.astype(mmnp)),
            "wqkvT": np.ascontiguousarray(w_loc.T.astype(mmnp)),
            "bqkvT": np.ascontiguousarray(b_loc.reshape(FT // P, P).T),
            "bv": np.ascontiguousarray(bqkv[vsl]),
            "w1T": np.ascontiguousarray(W1[:, hg * FQ:(hg + 1) * FQ].T.astype(mmnp)),
        })
    return in_maps


def combine_outputs(results, b1):
    b1 = np.asarray(b1, dtype=np.float32)
    out = np.empty((B, N, DIM), dtype=np.float32)
    for b in range(B):
        acc = (results[HG * b]["outT"].astype(np.float32)
               + results[HG * b + 1]["outT"].astype(np.float32))
        out[b] = acc.T + b1
    return out


def kernel(x, Wqkv, bqkv, W1, b1, trace=False):
    from concourse.bass_utils import run_bass_kernel_spmd

    nc = _get_nc()
    in_maps = make_in_maps(x, Wqkv, bqkv, W1)
    res = run_bass_kernel_spmd(
        nc, in_maps, core_ids=list(range(NCORES)), trace=trace
    )
    out = combine_outputs(res.results, b1)
    if trace:
        kernel.last_result = res
    return out
